# revision 3
# baseline (speedup 1.0000x reference)
# Multi-head self-attention (B=2, S=4096, D=512, H=8) on 8 NeuronCores. v2.
#
# Sharding: core c -> batch b = c//4, head-pair hp = c%4 (heads 2hp, 2hp+1).
# Host pre-slices/transposes weights + x per core; device does everything;
# host sums the 4 per-core W_O partials per batch and transposes back.
#
# v2 changes vs the fp16 baseline (306 us):
#  * AV matmul in fp8e4 (IEEE e4m3) with MatmulPerfMode.DoubleRow: each
#    instruction contracts 2x128 kpos at 0.5 cycles/col (4x fp16 MAC rate).
#    Accuracy is kept with an error-feedback split V = V8 + Vres8: two
#    interleaved DR chains accumulate into one PSUM bank (A: [V8|ones|0pad]
#    M=128 incl. the denominator column, B: Vres M=64, start only on A0,
#    stop only on the last A). Sim: rel 1.3-1.6e-2 vs 2e-2 budget.
#  * exp outputs e4m3 directly. The affine for the fp8 Schraudolph bit
#    trick is folded INTO the scores matmul: wk is pre-scaled by log2(e)
#    host-side and a constant B-row (extra contraction row) adds the bit
#    bias, so scores psum = 1.4427*s + B8. The DVE/Pool path is then just
#    round(clamp(psum, 0, 119)) -> int8 (one tensor_scalar, both ALU slots
#    used for the clamp; 119 = 0x77 = 240.0, the IEEE-e4m3 max — codes
#    120+ are inf/nan). The ACT path inverts the affine with scale/bias
#    inside the Exp activation. exp work is split ACT/DVE/Pool ~42/32/26
#    by a deficit-weighted round-robin to balance engine busy time.
#  * scores/projections/W_O stay fp16 (fp8 there fails the error budget;
#    DR+residual for scores costs more than fp16).
#  * yT output in fp16 (host accumulates partials in fp32).

import numpy as np

B, S, D, H, DK = 2, 4096, 512, 8, 64
P = 128          # partition tile
NQ = 512         # matmul moving free dim (one fp32 PSUM bank)
QCH = 1024       # q-chunk (2 x NQ)
NKC = S // P     # kpos chunks (32)
NPAIR = NKC // 2  # DoubleRow kpos chunk-pairs (16)
NST = S // NQ    # s-tiles of 512 (8)
NDC = D // P     # d chunks (4)
NQC = S // QCH   # q chunks (4)

# exp(s/8 - SHIFT) in e4m3: bits = round(clamp(KS*s + B8, 0, 119)).
# KS is folded into wk host-side; B8 enters via a constant contraction row.
SHIFT = 2.0
SCH_C = 0.055                      # centers the linear-mantissa error
KSCALE = 1.4426950408889634        # 8*log2(e)/8
A8 = 8.0 * KSCALE                  # e4m3 codes per e-fold
B8 = 56.0 - 8.0 * SCH_C - A8 * SHIFT       # 32.4768792...
ACT_SCALE = 1.0 / A8               # invert the affine for the true-exp path
ACT_BIAS = -(B8 / A8 + SHIFT)      # exp(psum*ACT_SCALE + ACT_BIAS)

# exp engine split (deficit-weighted round-robin). GPSIMD cannot read PSUM,
# so only ACT/DVE can consume the scores; Pool gets SBUF-only side work.
ENG_FRAC = {"act": 0.557, "dve": 0.443}

TRACE = False            # test.py sets True to get exec_time_ns + perfetto
TMPDIR = None            # optional trace output dir
LAST_RESULTS = None      # BassKernelResults of the last run (for test.py)
EXP_ASSIGN = []          # engine per exp op, emission order (for sim model)

_CACHE = {}


def _build_nc():
    import concourse.bass as bass  # noqa: F401
    import concourse.mybir as mybir
    import concourse.tile as tile
    from concourse import bacc

    f32 = mybir.dt.float32
    f16 = mybir.dt.float16
    f8 = mybir.dt.float8e4
    i8 = mybir.dt.int8
    Act = mybir.ActivationFunctionType
    Alu = mybir.AluOpType
    DR = mybir.MatmulPerfMode.DoubleRow

    nc = bacc.Bacc("TRN2", target_bir_lowering=False, debug=False, num_devices=8)

    xT = nc.dram_tensor("xT", [D, S], f16, kind="ExternalInput")
    wqS = nc.dram_tensor("wqS", [P, D], f16, kind="ExternalInput")
    wkS = nc.dram_tensor("wkS", [P, D], f16, kind="ExternalInput")  # *KSCALE
    wvS = nc.dram_tensor("wvS", [P, D], f16, kind="ExternalInput")
    woT0 = nc.dram_tensor("woT0", [DK, D], f16, kind="ExternalInput")
    woT1 = nc.dram_tensor("woT1", [DK, D], f16, kind="ExternalInput")
    yT = nc.dram_tensor("yT", [D, S], f16, kind="ExternalOutput")

    # deficit round-robin over the 512 exp ops
    eng_done = {k: 0.0 for k in ENG_FRAC}
    eng_n = [0]

    def pick_engine():
        eng_n[0] += 1
        best, bestd = None, None
        for k, f in ENG_FRAC.items():
            d = f * eng_n[0] - eng_done[k]
            if bestd is None or d > bestd:
                best, bestd = k, d
        eng_done[best] += 1.0
        EXP_ASSIGN.append(best)
        return best

    with tile.TileContext(nc) as tc:
        with (
            tc.tile_pool(name="sb", bufs=1) as sb,
            tc.tile_pool(name="ps", bufs=1, space="PSUM") as psp,
        ):
            # PSUM budget (8 banks): sc{h}{sub} 4 x [128,512] + av/phase1/WO
            # slots av0..av3 shared by tag rotation.
            psn = 0

            def av_ps(shape):
                nonlocal psn
                t = psp.tile(shape, f32, tag=f"av{psn % 4}", name=f"avps{psn}")
                psn += 1
                return t

            # ---- persistent operand tiles -----------------------------------
            qtz = [sb.tile([P, S], f16, tag=f"qtz{h}", name=f"qtz{h}")
                   for h in range(2)]
            ktz = [sb.tile([P, S], f16, tag=f"ktz{h}", name=f"ktz{h}")
                   for h in range(2)]
            # A-chain stationary: [kpos, chunk, 64 V8 | ones | 63 zeros]
            vb8a = [sb.tile([P, NKC, P], f8, tag=f"vb8a{h}", name=f"vb8a{h}")
                    for h in range(2)]
            # B-chain stationary: Vres8
            vb8r = [sb.tile([P, NKC, DK], f8, tag=f"vb8r{h}", name=f"vb8r{h}")
                    for h in range(2)]
            outtz = [sb.tile([P, S], f16, tag=f"outtz{h}", name=f"outtz{h}")
                     for h in range(2)]
            wosz = [sb.tile([P, D], f16, tag=f"wosz{h}", name=f"wosz{h}")
                    for h in range(2)]

            # head h=0 lives on partitions 0:64 (B-row at 64, zeros above),
            # head h=1 on partitions 64:128 (B-row at 0, zeros below).
            # DVE for the scores operands (startup critical path), gpsimd for
            # the rest.
            nc.vector.memset(qtz[0][DK:P, :], 0.0)
            nc.vector.memset(qtz[0][DK:DK + 1, :], 1.0)
            nc.vector.memset(ktz[0][DK:P, :], 0.0)
            nc.vector.memset(ktz[0][DK:DK + 1, :], B8)
            nc.vector.memset(qtz[1][0:DK, :], 0.0)
            nc.vector.memset(qtz[1][0:1, :], 1.0)
            nc.vector.memset(ktz[1][0:DK, :], 0.0)
            nc.vector.memset(ktz[1][0:1, :], B8)
            for h in range(2):
                # ones col 64, zero pad 65:128 (V8 cols written by phase 1)
                nc.gpsimd.memset(vb8a[h][:, :, DK:DK + 1], 1.0)
                nc.gpsimd.memset(vb8a[h][:, :, DK + 1:P], 0.0)
                nc.gpsimd.memset(outtz[h][DK:P, :], 0.0)
                nc.gpsimd.memset(wosz[h][DK:P, :], 0.0)

            # bias const for the ACT exp path
            biast = sb.tile([P, 1], f32, tag="biast", name="biast")
            nc.gpsimd.memset(biast[:, :], ACT_BIAS)

            # ---- phase 1: load x + weights, QKV projections, build V -------
            xts = [sb.tile([P, S], f16, tag=f"xt{dc}", name=f"xt{dc}")
                   for dc in range(NDC)]
            wsb = {}
            for name in ("v", "k", "q"):
                wsb[name] = sb.tile([P, NDC * P], f16, tag=f"w{name}",
                                    name=f"w{name}")
            # weights first (small) so the first projection matmul isn't
            # gated behind the 4MB x stream; then x quarter-tiles
            for name, dram in (("v", wvS), ("k", wkS), ("q", wqS)):
                nc.sync.dma_start(wsb[name][:, :], dram[:, :])
            for quart in range(4):
                hs = slice(quart * (S // 4), (quart + 1) * (S // 4))
                for dc in range(NDC):
                    nc.sync.dma_start(xts[dc][:, hs], xT[dc * P:(dc + 1) * P, hs])
            nc.sync.dma_start(wosz[0][0:DK, :], woT0[:, :])
            nc.sync.dma_start(wosz[1][0:DK, :], woT1[:, :])

            for st in range(NST):
                for name in ("v", "k", "q"):
                    w = wsb[name]
                    if name == "v":
                        # V directly in [kpos, chan] layout; 4 chunks per st
                        # into one 3-D psum bank, then fp8 cast + residual
                        vps = av_ps([P, 4, P])
                        for c4 in range(4):
                            ch = st * 4 + c4
                            for dc in range(NDC):
                                nc.tensor.matmul(
                                    vps[:, c4:c4 + 1, :],
                                    xts[dc][:, ch * P:(ch + 1) * P],
                                    w[:, dc * P:(dc + 1) * P],
                                    start=(dc == 0),
                                    stop=(dc == NDC - 1),
                                )
                        cs = slice(st * 4, (st + 1) * 4)
                        for h in range(2):
                            hsl = slice(h * DK, (h + 1) * DK)
                            nc.scalar.copy(
                                vb8a[h][:, cs, 0:DK], vps[:, :, hsl]
                            )
                            nc.vector.tensor_sub(
                                vb8r[h][:, cs, :], vps[:, :, hsl],
                                vb8a[h][:, cs, 0:DK],
                            )
                        continue
                    ps = av_ps([P, NQ])
                    for dc in range(NDC):
                        nc.tensor.matmul(
                            ps[:, :],
                            w[:, dc * P:(dc + 1) * P],
                            xts[dc][:, st * NQ:(st + 1) * NQ],
                            start=(dc == 0),
                            stop=(dc == NDC - 1),
                        )
                    sl = slice(st * NQ, (st + 1) * NQ)
                    if name == "k":
                        nc.scalar.copy(ktz[0][0:DK, sl], ps[0:DK, :])
                        nc.scalar.copy(ktz[1][DK:P, sl], ps[DK:P, :])
                    else:
                        nc.vector.tensor_copy(qtz[0][0:DK, sl], ps[0:DK, :])
                        nc.vector.tensor_copy(qtz[1][DK:P, sl], ps[DK:P, :])

            # ---- phase 2: flash attention -----------------------------------
            def emit_normalize(qc, av):
                for sub in range(2):
                    for h in range(2):
                        a = av[h, sub]
                        raw = sb.tile([DK + 1, NQ], f32, tag=f"raw{h}{sub}",
                                      name=f"raw{h}{sub}", bufs=3)
                        nc.vector.tensor_copy(raw[:, :], a[0:DK + 1, :])
                        dn0 = sb.tile([P, NQ], f32, tag="dn0", bufs=4)
                        nc.sync.dma_start(dn0[0:1, :], raw[DK:DK + 1, :])
                        rc = sb.tile([P, NQ], f32, tag="rc", bufs=4)
                        nc.vector.reciprocal_approx_fast(rc[0:1, :], dn0[0:1, :])
                        rcb = sb.tile([DK, NQ], f32, tag="rcb", bufs=4)
                        nc.gpsimd.partition_broadcast(
                            rcb[:, :], rc[0:1, :], channels=DK
                        )
                        q0 = qc * QCH + sub * NQ
                        nc.gpsimd.tensor_mul(
                            outtz[h][0:DK, q0:q0 + NQ], raw[0:DK, :], rcb[:, :]
                        )
                    emit_wo_st(2 * qc + sub)

            def emit_wo_st(st):
                for ec in range(NDC):
                    yp = av_ps([P, NQ])
                    for h in range(2):
                        nc.tensor.matmul(
                            yp[:, :],
                            wosz[h][:, ec * P:(ec + 1) * P],
                            outtz[h][:, st * NQ:(st + 1) * NQ],
                            start=(h == 0),
                            stop=(h == 1),
                        )
                    ys = sb.tile([P, NQ], f16, tag="ys", bufs=4)
                    if st >= NST - 2:
                        nc.vector.tensor_copy(ys[:, :], yp[:, :])
                    else:
                        nc.scalar.copy(ys[:, :], yp[:, :])
                    nc.sync.dma_start(
                        yT[ec * P:(ec + 1) * P, st * NQ:(st + 1) * NQ],
                        ys[:, :],
                    )

            pending = None
            av_pending = None

            def emit_av_pair(j, exb):
                # interleaved A (V8+ones, M=128) / B (Vres, M=64) DR chains
                # accumulating into the same bank; start on A0, stop on the
                # last chain instruction.
                last = j == NPAIR - 1
                for h in range(2):
                    for sub in range(2):
                        rhs = exb[h][:, :, sub * NQ:(sub + 1) * NQ]
                        a = av[h, sub]
                        nc.tensor.matmul(
                            a[:, :], vb8a[h][:, 2 * j:2 * j + 2, :], rhs,
                            start=(j == 0), stop=False,
                            perf_mode=DR, skip_group_check=True,
                        )
                        nc.tensor.matmul(
                            a[0:DK, :], vb8r[h][:, 2 * j:2 * j + 2, :], rhs,
                            start=False, stop=last,
                            perf_mode=DR, skip_group_check=True,
                        )

            for qc in range(NQC):
                av = {}
                for h in range(2):
                    for sub in range(2):
                        av[h, sub] = av_ps([P, NQ])
                exb = None
                for k in range(NKC):
                    if k == 3 and pending is not None:
                        emit_normalize(*pending)
                        pending = None
                    j, par = divmod(k, 2)
                    scps = [[psp.tile([P, NQ], f32, tag=f"sc{h}{sub}",
                                      name=f"sc{h}{sub}")
                             for sub in range(2)] for h in range(2)]
                    for h in range(2):
                        for sub in range(2):
                            q0 = qc * QCH + sub * NQ
                            nc.tensor.matmul(
                                scps[h][sub][:, :],
                                ktz[h][:, k * P:(k + 1) * P],
                                qtz[h][:, q0:q0 + NQ],
                                start=True,
                                stop=True,
                            )
                    if par == 0:
                        exb = {h: sb.tile([P, 2, QCH], f8, tag=f"ex{h}",
                                          name=f"ex{h}", bufs=3)
                               for h in range(2)}
                    for h in range(2):
                        exi8 = exb[h].bitcast(i8)
                        for sub in range(2):
                            ssl = slice(sub * NQ, (sub + 1) * NQ)
                            eng = pick_engine()
                            if eng == "act":
                                nc.scalar.activation(
                                    exb[h][:, par:par + 1, ssl],
                                    scps[h][sub][:, :], Act.Exp,
                                    scale=ACT_SCALE, bias=biast[:, 0:1],
                                )
                            else:
                                nc.vector.tensor_scalar(
                                    exi8[:, par:par + 1, ssl],
                                    scps[h][sub][:, :], 0.0, 119.0,
                                    Alu.max, Alu.min,
                                )
                    if par == 1:
                        if av_pending is not None:
                            emit_av_pair(*av_pending)
                        av_pending = (j, exb)
                emit_av_pair(*av_pending)
                av_pending = None
                pending = (qc, av)
            emit_normalize(*pending)

    nc.compile()
    return nc


def _wsb(w, e0, scale=1.0):
    # SBUF weight layout: w_sb[p, dc*P + m] = w[e0 + m, dc*P + p]
    wt = (w[e0:e0 + P].T * scale).astype(np.float16)     # [D, P]
    return np.ascontiguousarray(
        np.hstack([wt[d * P:(d + 1) * P, :] for d in range(NDC)])
    )


def kernel(x, wq, wk, wv, wo):
    global LAST_RESULTS
    from concourse.bass_utils import run_bass_kernel_spmd

    if "nc" not in _CACHE:
        _CACHE["nc"] = _build_nc()
    nc = _CACHE["nc"]

    x = np.asarray(x, dtype=np.float32)
    wq = np.asarray(wq, dtype=np.float32)
    wk = np.asarray(wk, dtype=np.float32)
    wv = np.asarray(wv, dtype=np.float32)
    wo = np.asarray(wo, dtype=np.float32)

    in_maps = []
    for c in range(8):
        b, hp = divmod(c, 4)
        e0 = hp * P
        in_maps.append({
            "xT": np.ascontiguousarray(x[b].T.astype(np.float16)),
            "wqS": _wsb(wq, e0),
            "wkS": _wsb(wk, e0, scale=KSCALE),
            "wvS": _wsb(wv, e0),
            "woT0": np.ascontiguousarray(wo[:, e0:e0 + DK].T.astype(np.float16)),
            "woT1": np.ascontiguousarray(wo[:, e0 + DK:e0 + P].T.astype(np.float16)),
        })

    res = run_bass_kernel_spmd(
        nc, in_maps, core_ids=list(range(8)), trace=TRACE, tmpdir=TMPDIR
    )
    LAST_RESULTS = res

    y = np.zeros((B, S, D), dtype=np.float32)
    for c in range(8):
        y[c // 4] += res.results[c]["yT"].T.astype(np.float32)
    return y


# revision 4
# speedup vs baseline: 1.2434x; 1.2434x over previous
# Multi-head self-attention (B=2, S=4096, D=512, H=8) on 8 NeuronCores. v2.
#
# Sharding: core c -> batch b = c//4, head-pair hp = c%4 (heads 2hp, 2hp+1).
# Host pre-slices/transposes weights + x per core; device does everything;
# host sums the 4 per-core W_O partials per batch and transposes back.
#
# v2 changes vs the fp16 baseline (306 us):
#  * AV matmul in fp8e4 (IEEE e4m3) with MatmulPerfMode.DoubleRow: each
#    instruction contracts 2x128 kpos at 0.5 cycles/col (4x fp16 MAC rate).
#    Accuracy is kept with an error-feedback split V = V8 + Vres8: two
#    interleaved DR chains accumulate into one PSUM bank (A: [V8|ones|0pad]
#    M=128 incl. the denominator column, B: Vres M=64, start only on A0,
#    stop only on the last A). Sim: rel 1.3-1.6e-2 vs 2e-2 budget.
#  * exp outputs e4m3 directly. The affine for the fp8 Schraudolph bit
#    trick is folded INTO the scores matmul: wk is pre-scaled by log2(e)
#    host-side and a constant B-row (extra contraction row) adds the bit
#    bias, so scores psum = 1.4427*s + B8. The DVE/Pool path is then just
#    round(clamp(psum, 0, 119)) -> int8 (one tensor_scalar, both ALU slots
#    used for the clamp; 119 = 0x77 = 240.0, the IEEE-e4m3 max — codes
#    120+ are inf/nan). The ACT path inverts the affine with scale/bias
#    inside the Exp activation. exp work is split ACT/DVE/Pool ~42/32/26
#    by a deficit-weighted round-robin to balance engine busy time.
#  * scores/projections/W_O stay fp16 (fp8 there fails the error budget;
#    DR+residual for scores costs more than fp16).
#  * yT output in fp16 (host accumulates partials in fp32).

import numpy as np

B, S, D, H, DK = 2, 4096, 512, 8, 64
P = 128          # partition tile
NQ = 512         # matmul moving free dim (one fp32 PSUM bank)
QCH = 1024       # q-chunk (2 x NQ)
NKC = S // P     # kpos chunks (32)
NPAIR = NKC // 2  # DoubleRow kpos chunk-pairs (16)
NST = S // NQ    # s-tiles of 512 (8)
NDC = D // P     # d chunks (4)
NQC = S // QCH   # q chunks (4)

# exp(s/8 - SHIFT) in e4m3: bits = round(clamp(KS*s + B8, 0, 119)).
# KS is folded into wk host-side; B8 enters via a constant contraction row.
SHIFT = 2.0
SCH_C = 0.055                      # centers the linear-mantissa error
KSCALE = 1.4426950408889634        # 8*log2(e)/8
A8 = 8.0 * KSCALE                  # e4m3 codes per e-fold
B8 = 56.0 - 8.0 * SCH_C - A8 * SHIFT       # 32.4768792...
ACT_SCALE = 1.0 / A8               # invert the affine for the true-exp path
ACT_BIAS = -(B8 / A8 + SHIFT)      # exp(psum*ACT_SCALE + ACT_BIAS)

# exp engine split (deficit-weighted round-robin). GPSIMD cannot read PSUM,
# so only ACT/DVE can consume the scores; Pool gets SBUF-only side work.
ENG_FRAC = {"act": 0.557, "dve": 0.443}

TRACE = False            # test.py sets True to get exec_time_ns + perfetto
TMPDIR = None            # optional trace output dir
LAST_RESULTS = None      # BassKernelResults of the last run (for test.py)
EXP_ASSIGN = []          # engine per exp op, emission order (for sim model)

_CACHE = {}


def _build_nc():
    import concourse.bass as bass  # noqa: F401
    import concourse.mybir as mybir
    import concourse.tile as tile
    from concourse import bacc

    f32 = mybir.dt.float32
    f16 = mybir.dt.float16
    f8 = mybir.dt.float8e4
    i8 = mybir.dt.int8
    Act = mybir.ActivationFunctionType
    Alu = mybir.AluOpType
    DR = mybir.MatmulPerfMode.DoubleRow

    nc = bacc.Bacc("TRN2", target_bir_lowering=False, debug=False, num_devices=8)

    xT = nc.dram_tensor("xT", [D, S], f16, kind="ExternalInput")
    wqS = nc.dram_tensor("wqS", [P, D], f16, kind="ExternalInput")
    wkS = nc.dram_tensor("wkS", [P, D], f16, kind="ExternalInput")  # *KSCALE
    wvS = nc.dram_tensor("wvS", [P, D], f16, kind="ExternalInput")
    woT0 = nc.dram_tensor("woT0", [DK, D], f16, kind="ExternalInput")
    woT1 = nc.dram_tensor("woT1", [DK, D], f16, kind="ExternalInput")
    yT = nc.dram_tensor("yT", [D, S], f16, kind="ExternalOutput")

    # deficit round-robin over the 512 exp ops
    eng_done = {k: 0.0 for k in ENG_FRAC}
    eng_n = [0]

    def pick_engine():
        eng_n[0] += 1
        best, bestd = None, None
        for k, f in ENG_FRAC.items():
            d = f * eng_n[0] - eng_done[k]
            if bestd is None or d > bestd:
                best, bestd = k, d
        eng_done[best] += 1.0
        EXP_ASSIGN.append(best)
        return best

    with tile.TileContext(nc) as tc:
        with (
            tc.tile_pool(name="sb", bufs=1) as sb,
            tc.tile_pool(name="ps", bufs=1, space="PSUM") as psp,
        ):
            # PSUM budget (8 banks): sc{h}{sub} 4 x [128,512] + av/phase1/WO
            # slots av0..av3 shared by tag rotation.
            psn = 0

            def av_ps(shape):
                nonlocal psn
                t = psp.tile(shape, f32, tag=f"av{psn % 4}", name=f"avps{psn}")
                psn += 1
                return t

            # ---- persistent operand tiles -----------------------------------
            qtz = [sb.tile([P, S], f16, tag=f"qtz{h}", name=f"qtz{h}")
                   for h in range(2)]
            ktz = [sb.tile([P, S], f16, tag=f"ktz{h}", name=f"ktz{h}")
                   for h in range(2)]
            # A-chain stationary: [kpos, chunk, 64 V8 | ones | 63 zeros]
            vb8a = [sb.tile([P, NKC, P], f8, tag=f"vb8a{h}", name=f"vb8a{h}")
                    for h in range(2)]
            # B-chain stationary: Vres8
            vb8r = [sb.tile([P, NKC, DK], f8, tag=f"vb8r{h}", name=f"vb8r{h}")
                    for h in range(2)]
            outtz = [sb.tile([P, S], f16, tag=f"outtz{h}", name=f"outtz{h}")
                     for h in range(2)]
            wosz = [sb.tile([P, D], f16, tag=f"wosz{h}", name=f"wosz{h}")
                    for h in range(2)]

            # head h=0 lives on partitions 0:64 (B-row at 64, zeros above),
            # head h=1 on partitions 64:128 (B-row at 0, zeros below).
            # DVE for the scores operands (startup critical path), gpsimd for
            # the rest.
            nc.vector.memset(qtz[0][DK:P, :], 0.0)
            nc.vector.memset(qtz[0][DK:DK + 1, :], 1.0)
            nc.vector.memset(ktz[0][DK:P, :], 0.0)
            nc.vector.memset(ktz[0][DK:DK + 1, :], B8)
            nc.vector.memset(qtz[1][0:DK, :], 0.0)
            nc.vector.memset(qtz[1][0:1, :], 1.0)
            nc.vector.memset(ktz[1][0:DK, :], 0.0)
            nc.vector.memset(ktz[1][0:1, :], B8)
            for h in range(2):
                # ones col 64, zero pad 65:128 (V8 cols written by phase 1)
                nc.gpsimd.memset(vb8a[h][:, :, DK:DK + 1], 1.0)
                nc.gpsimd.memset(vb8a[h][:, :, DK + 1:P], 0.0)
                nc.gpsimd.memset(outtz[h][DK:P, :], 0.0)
                nc.gpsimd.memset(wosz[h][DK:P, :], 0.0)

            # bias const for the ACT exp path
            biast = sb.tile([P, 1], f32, tag="biast", name="biast")
            nc.gpsimd.memset(biast[:, :], ACT_BIAS)

            # ---- phase 1: load x + weights, QKV projections, build V -------
            xts = [sb.tile([P, S], f16, tag=f"xt{dc}", name=f"xt{dc}")
                   for dc in range(NDC)]
            wsb = {}
            for name in ("v", "k", "q"):
                wsb[name] = sb.tile([P, NDC * P], f16, tag=f"w{name}",
                                    name=f"w{name}")
            # weights first (small) so the first projection matmul isn't
            # gated behind the 4MB x stream; then x quarter-tiles
            for name, dram in (("v", wvS), ("k", wkS), ("q", wqS)):
                nc.sync.dma_start(wsb[name][:, :], dram[:, :])
            for quart in range(4):
                hs = slice(quart * (S // 4), (quart + 1) * (S // 4))
                for dc in range(NDC):
                    nc.sync.dma_start(xts[dc][:, hs], xT[dc * P:(dc + 1) * P, hs])
            nc.sync.dma_start(wosz[0][0:DK, :], woT0[:, :])
            nc.sync.dma_start(wosz[1][0:DK, :], woT1[:, :])

            for st in range(NST):
                for name in ("v", "k", "q"):
                    w = wsb[name]
                    if name == "v":
                        # V directly in [kpos, chan] layout; 4 chunks per st
                        # into one 3-D psum bank, then fp8 cast + residual
                        vps = av_ps([P, 4, P])
                        for c4 in range(4):
                            ch = st * 4 + c4
                            for dc in range(NDC):
                                nc.tensor.matmul(
                                    vps[:, c4:c4 + 1, :],
                                    xts[dc][:, ch * P:(ch + 1) * P],
                                    w[:, dc * P:(dc + 1) * P],
                                    start=(dc == 0),
                                    stop=(dc == NDC - 1),
                                )
                        cs = slice(st * 4, (st + 1) * 4)
                        for h in range(2):
                            hsl = slice(h * DK, (h + 1) * DK)
                            nc.scalar.copy(
                                vb8a[h][:, cs, 0:DK], vps[:, :, hsl]
                            )
                            nc.vector.tensor_sub(
                                vb8r[h][:, cs, :], vps[:, :, hsl],
                                vb8a[h][:, cs, 0:DK],
                            )
                        continue
                    ps = av_ps([P, NQ])
                    for dc in range(NDC):
                        nc.tensor.matmul(
                            ps[:, :],
                            w[:, dc * P:(dc + 1) * P],
                            xts[dc][:, st * NQ:(st + 1) * NQ],
                            start=(dc == 0),
                            stop=(dc == NDC - 1),
                        )
                    sl = slice(st * NQ, (st + 1) * NQ)
                    if name == "k":
                        nc.scalar.copy(ktz[0][0:DK, sl], ps[0:DK, :])
                        nc.scalar.copy(ktz[1][DK:P, sl], ps[DK:P, :])
                    else:
                        nc.vector.tensor_copy(qtz[0][0:DK, sl], ps[0:DK, :])
                        nc.vector.tensor_copy(qtz[1][DK:P, sl], ps[DK:P, :])

            # ---- phase 2: flash attention -----------------------------------
            def emit_normalize(qc, av):
                # stage-parallel across the 4 (h,sub): evacs, then DMAs, then
                # recips, then the (batched, one ucode library) broadcasts,
                # then muls — pipelines instead of 4 serial latency chains
                raws, rcs, rcbs = {}, {}, {}
                for sub in range(2):
                    for h in range(2):
                        raw = sb.tile([DK + 1, NQ], f32, tag=f"raw{h}{sub}",
                                      name=f"raw{h}{sub}", bufs=3)
                        nc.vector.tensor_copy(raw[:, :], av[h, sub][0:DK + 1, :])
                        raws[h, sub] = raw
                for sub in range(2):
                    for h in range(2):
                        dn0 = sb.tile([P, NQ], f32, tag=f"dn{h}{sub}", bufs=2)
                        nc.sync.dma_start(dn0[0:1, :], raws[h, sub][DK:DK + 1, :])
                        rc = sb.tile([P, NQ], f32, tag=f"rc{h}{sub}", bufs=2)
                        nc.vector.reciprocal_approx_fast(rc[0:1, :], dn0[0:1, :])
                        rcs[h, sub] = rc
                for sub in range(2):
                    for h in range(2):
                        rcb = sb.tile([DK, NQ], f32, tag=f"rcb{h}{sub}", bufs=2)
                        nc.gpsimd.partition_broadcast(
                            rcb[:, :], rcs[h, sub][0:1, :], channels=DK
                        )
                        rcbs[h, sub] = rcb
                for sub in range(2):
                    for h in range(2):
                        q0 = qc * QCH + sub * NQ
                        nc.vector.tensor_mul(
                            outtz[h][0:DK, q0:q0 + NQ], raws[h, sub][0:DK, :],
                            rcbs[h, sub][:, :],
                        )
                for sub in range(2):
                    emit_wo_st(2 * qc + sub)

            def emit_wo_st(st):
                for ec in range(NDC):
                    yp = av_ps([P, NQ])
                    for h in range(2):
                        nc.tensor.matmul(
                            yp[:, :],
                            wosz[h][:, ec * P:(ec + 1) * P],
                            outtz[h][:, st * NQ:(st + 1) * NQ],
                            start=(h == 0),
                            stop=(h == 1),
                        )
                    ys = sb.tile([P, NQ], f16, tag="ys", bufs=4)
                    if st >= NST - 2:
                        nc.vector.tensor_copy(ys[:, :], yp[:, :])
                    else:
                        nc.scalar.copy(ys[:, :], yp[:, :])
                    nc.sync.dma_start(
                        yT[ec * P:(ec + 1) * P, st * NQ:(st + 1) * NQ],
                        ys[:, :],
                    )

            pending = None
            av_pending = None

            def emit_av_pair(j, exb):
                # interleaved A (V8+ones, M=128) / B (Vres, M=64) DR chains
                # accumulating into the same bank; start on A0, stop on the
                # last chain instruction.
                last = j == NPAIR - 1
                for h in range(2):
                    for sub in range(2):
                        rhs = exb[h][:, :, sub * NQ:(sub + 1) * NQ]
                        a = av[h, sub]
                        nc.tensor.matmul(
                            a[:, :], vb8a[h][:, 2 * j:2 * j + 2, :], rhs,
                            start=(j == 0), stop=False,
                            perf_mode=DR, skip_group_check=True,
                        )
                        nc.tensor.matmul(
                            a[0:DK, :], vb8r[h][:, 2 * j:2 * j + 2, :], rhs,
                            start=False, stop=last,
                            perf_mode=DR, skip_group_check=True,
                        )

            for qc in range(NQC):
                # av tiles are claimed AFTER the pending normalize's W_O yp
                # tiles (at k==3) so the tag-ring WAR chain is
                # av(qc-1) -> wo(qc-1) -> av(qc), not av(qc) -> wo(qc-1):
                # otherwise W_O(qc-1) waits a full extra qc and the whole
                # exp->AV pipeline clogs behind it at every qc boundary.
                av = {}

                def alloc_av():
                    for h in range(2):
                        for sub in range(2):
                            av[h, sub] = av_ps([P, NQ])

                if pending is None:
                    alloc_av()
                exb = None
                for k in range(NKC):
                    if k == 3 and pending is not None:
                        emit_normalize(*pending)
                        pending = None
                        alloc_av()
                    j, par = divmod(k, 2)
                    scps = [[psp.tile([P, NQ], f32, tag=f"sc{h}{sub}",
                                      name=f"sc{h}{sub}")
                             for sub in range(2)] for h in range(2)]
                    for h in range(2):
                        for sub in range(2):
                            q0 = qc * QCH + sub * NQ
                            nc.tensor.matmul(
                                scps[h][sub][:, :],
                                ktz[h][:, k * P:(k + 1) * P],
                                qtz[h][:, q0:q0 + NQ],
                                start=True,
                                stop=True,
                            )
                    if par == 0:
                        exb = {h: sb.tile([P, 2, QCH], f8, tag=f"ex{h}",
                                          name=f"ex{h}", bufs=4)
                               for h in range(2)}
                    for h in range(2):
                        exi8 = exb[h].bitcast(i8)
                        for sub in range(2):
                            ssl = slice(sub * NQ, (sub + 1) * NQ)
                            eng = pick_engine()
                            if eng == "act":
                                nc.scalar.activation(
                                    exb[h][:, par:par + 1, ssl],
                                    scps[h][sub][:, :], Act.Exp,
                                    scale=ACT_SCALE, bias=biast[:, 0:1],
                                )
                            else:
                                nc.vector.tensor_scalar(
                                    exi8[:, par:par + 1, ssl],
                                    scps[h][sub][:, :], 0.0, 119.0,
                                    Alu.max, Alu.min,
                                )
                    if par == 1:
                        if av_pending is not None:
                            emit_av_pair(*av_pending)
                        av_pending = (j, exb)
                emit_av_pair(*av_pending)
                av_pending = None
                pending = (qc, av)
            emit_normalize(*pending)

    nc.compile()
    return nc


def _wsb(w, e0, scale=1.0):
    # SBUF weight layout: w_sb[p, dc*P + m] = w[e0 + m, dc*P + p]
    wt = (w[e0:e0 + P].T * scale).astype(np.float16)     # [D, P]
    return np.ascontiguousarray(
        np.hstack([wt[d * P:(d + 1) * P, :] for d in range(NDC)])
    )


def kernel(x, wq, wk, wv, wo):
    global LAST_RESULTS
    from concourse.bass_utils import run_bass_kernel_spmd

    if "nc" not in _CACHE:
        _CACHE["nc"] = _build_nc()
    nc = _CACHE["nc"]

    x = np.asarray(x, dtype=np.float32)
    wq = np.asarray(wq, dtype=np.float32)
    wk = np.asarray(wk, dtype=np.float32)
    wv = np.asarray(wv, dtype=np.float32)
    wo = np.asarray(wo, dtype=np.float32)

    in_maps = []
    for c in range(8):
        b, hp = divmod(c, 4)
        e0 = hp * P
        in_maps.append({
            "xT": np.ascontiguousarray(x[b].T.astype(np.float16)),
            "wqS": _wsb(wq, e0),
            "wkS": _wsb(wk, e0, scale=KSCALE),
            "wvS": _wsb(wv, e0),
            "woT0": np.ascontiguousarray(wo[:, e0:e0 + DK].T.astype(np.float16)),
            "woT1": np.ascontiguousarray(wo[:, e0 + DK:e0 + P].T.astype(np.float16)),
        })

    res = run_bass_kernel_spmd(
        nc, in_maps, core_ids=list(range(8)), trace=TRACE, tmpdir=TMPDIR
    )
    LAST_RESULTS = res

    y = np.zeros((B, S, D), dtype=np.float32)
    for c in range(8):
        y[c // 4] += res.results[c]["yT"].T.astype(np.float32)
    return y


# revision 5
# speedup vs baseline: 1.6766x; 1.3485x over previous
# Multi-head self-attention (B=2, S=4096, D=512, H=8) on 8 NeuronCores. v2.
#
# Sharding: core c -> batch b = c//4, head-pair hp = c%4 (heads 2hp, 2hp+1).
# Host pre-slices/transposes weights + x per core; device does everything;
# host sums the 4 per-core W_O partials per batch and transposes back.
#
# v2 changes vs the fp16 baseline (306 us):
#  * AV matmul in fp8e4 (IEEE e4m3) with MatmulPerfMode.DoubleRow: each
#    instruction contracts 2x128 kpos (matmul cost is N-moving-cols bound,
#    so doubling K halves the AV instruction count). Accuracy is kept with
#    an error-feedback split V = V8 + Vres8 packed into ONE stationary:
#    columns [V8(64) | ones(1) | Vres(dk 0:62)] (M=128). PSUM rows 0:63 =
#    V8-AV, row 64 = denominator, rows 65:128 = residual-AV; normalize
#    folds the residual back with a partition-shifting DMA + DVE add.
#    (dk63 keeps plain-V8 accuracy: 1 of 64 channels, ~1e-3 metric cost.)
#  * exp outputs e4m3 directly. The affine for the fp8 Schraudolph bit
#    trick is folded INTO the scores matmul: wk is pre-scaled by log2(e)
#    host-side and a constant B-row (extra contraction row) adds the bit
#    bias, so scores psum = 1.4427*s + B8. The DVE/Pool path is then just
#    round(clamp(psum, 0, 119)) -> int8 (one tensor_scalar, both ALU slots
#    used for the clamp; 119 = 0x77 = 240.0, the IEEE-e4m3 max — codes
#    120+ are inf/nan). The ACT path inverts the affine with scale/bias
#    inside the Exp activation. exp work is split ACT/DVE/Pool ~42/32/26
#    by a deficit-weighted round-robin to balance engine busy time.
#  * scores/projections/W_O stay fp16 (fp8 there fails the error budget;
#    DR+residual for scores costs more than fp16).
#  * yT output in fp16 (host accumulates partials in fp32).

import numpy as np

B, S, D, H, DK = 2, 4096, 512, 8, 64
P = 128          # partition tile
NQ = 512         # matmul moving free dim (one fp32 PSUM bank)
QCH = 1024       # q-chunk (2 x NQ)
NKC = S // P     # kpos chunks (32)
NPAIR = NKC // 2  # DoubleRow kpos chunk-pairs (16)
NST = S // NQ    # s-tiles of 512 (8)
NDC = D // P     # d chunks (4)
NQC = S // QCH   # q chunks (4)

# exp(s/8 - SHIFT) in e4m3: bits = round(clamp(KS*s + B8, 0, 119)).
# KS is folded into wk host-side; B8 enters via a constant contraction row.
SHIFT = 2.0
SCH_C = 0.055                      # centers the linear-mantissa error
KSCALE = 1.4426950408889634        # 8*log2(e)/8
A8 = 8.0 * KSCALE                  # e4m3 codes per e-fold
B8 = 56.0 - 8.0 * SCH_C - A8 * SHIFT       # 32.4768792...
ACT_SCALE = 1.0 / A8               # invert the affine for the true-exp path
ACT_BIAS = -(B8 / A8 + SHIFT)      # exp(psum*ACT_SCALE + ACT_BIAS)

# exp engine split (deficit-weighted round-robin). GPSIMD cannot read PSUM,
# so only ACT/DVE can consume the scores; Pool gets SBUF-only side work.
ENG_FRAC = {"act": 0.557, "dve": 0.443}

TRACE = False            # test.py sets True to get exec_time_ns + perfetto
TMPDIR = None            # optional trace output dir
LAST_RESULTS = None      # BassKernelResults of the last run (for test.py)
EXP_ASSIGN = []          # engine per exp op, emission order (for sim model)

_CACHE = {}


def _build_nc():
    import concourse.bass as bass  # noqa: F401
    import concourse.mybir as mybir
    import concourse.tile as tile
    from concourse import bacc

    f32 = mybir.dt.float32
    f16 = mybir.dt.float16
    f8 = mybir.dt.float8e4
    i8 = mybir.dt.int8
    Act = mybir.ActivationFunctionType
    Alu = mybir.AluOpType
    DR = mybir.MatmulPerfMode.DoubleRow

    nc = bacc.Bacc("TRN2", target_bir_lowering=False, debug=False, num_devices=8)

    xT = nc.dram_tensor("xT", [D, S], f16, kind="ExternalInput")
    wqS = nc.dram_tensor("wqS", [P, D], f16, kind="ExternalInput")
    wkS = nc.dram_tensor("wkS", [P, D], f16, kind="ExternalInput")  # *KSCALE
    wvS = nc.dram_tensor("wvS", [P, D], f16, kind="ExternalInput")
    woS = nc.dram_tensor("woS", [P, D], f16, kind="ExternalInput")  # stacked
    yT = nc.dram_tensor("yT", [D, S], f16, kind="ExternalOutput")

    # deficit round-robin over the 512 exp ops
    eng_done = {k: 0.0 for k in ENG_FRAC}
    eng_n = [0]

    def pick_engine():
        eng_n[0] += 1
        best, bestd = None, None
        for k, f in ENG_FRAC.items():
            d = f * eng_n[0] - eng_done[k]
            if bestd is None or d > bestd:
                best, bestd = k, d
        eng_done[best] += 1.0
        EXP_ASSIGN.append(best)
        return best

    with tile.TileContext(nc) as tc:
        with (
            tc.tile_pool(name="sb", bufs=1) as sb,
            tc.tile_pool(name="ps", bufs=1, space="PSUM") as psp,
        ):
            # PSUM budget (8 banks): sc{h}{sub} 4 x [128,512] + av/phase1/WO
            # slots av0..av3 shared by tag rotation.
            psn = 0

            def av_ps(shape):
                nonlocal psn
                t = psp.tile(shape, f32, tag=f"av{psn % 4}", name=f"avps{psn}")
                psn += 1
                return t

            # ---- persistent operand tiles -----------------------------------
            qtz = [sb.tile([P, S], f16, tag=f"qtz{h}", name=f"qtz{h}")
                   for h in range(2)]
            ktz = [sb.tile([P, S], f16, tag=f"ktz{h}", name=f"ktz{h}")
                   for h in range(2)]
            # AV stationary: [kpos, chunk, (V8 dk 0:64 | ones | Vres dk 0:63)]
            vb8a = [sb.tile([P, NKC, P], f8, tag=f"vb8a{h}", name=f"vb8a{h}")
                    for h in range(2)]
            # stacked: rows 0:64 head0, rows 64:128 head1
            outtz = sb.tile([P, S], f16, tag="outtz", name="outtz")
            wosz = sb.tile([P, D], f16, tag="wosz", name="wosz")

            # head h=0 lives on partitions 0:64 (B-row at 64, zeros above),
            # head h=1 on partitions 64:128 (B-row at 0, zeros below).
            # All on gpsimd: it is otherwise idle at startup, and DVE must
            # stay free for the phase-1 Q evacuations.
            nc.gpsimd.memset(qtz[0][DK:P, :], 0.0)
            nc.gpsimd.memset(qtz[0][DK:DK + 1, :], 1.0)
            nc.gpsimd.memset(ktz[0][DK:P, :], 0.0)
            nc.gpsimd.memset(ktz[0][DK:DK + 1, :], B8)
            nc.gpsimd.memset(qtz[1][0:DK, :], 0.0)
            nc.gpsimd.memset(qtz[1][0:1, :], 1.0)
            nc.gpsimd.memset(ktz[1][0:DK, :], 0.0)
            nc.gpsimd.memset(ktz[1][0:1, :], B8)
            for h in range(2):
                # ones col 64 (V8/Vres cols written by phase 1)
                nc.gpsimd.memset(vb8a[h][:, :, DK:DK + 1], 1.0)

            # bias const for the ACT exp path
            biast = sb.tile([P, 1], f32, tag="biast", name="biast")
            nc.gpsimd.memset(biast[:, :], ACT_BIAS)

            # ---- phase 1: load x + weights, QKV projections, build V -------
            xts = [sb.tile([P, S], f16, tag=f"xt{dc}", name=f"xt{dc}")
                   for dc in range(NDC)]
            wsb = {}
            for name in ("v", "k", "q"):
                wsb[name] = sb.tile([P, NDC * P], f16, tag=f"w{name}",
                                    name=f"w{name}")
            # weights first (small) so the first projection matmul isn't
            # gated behind the 4MB x stream; then x quarter-tiles
            for name, dram in (("v", wvS), ("k", wkS), ("q", wqS)):
                nc.sync.dma_start(wsb[name][:, :], dram[:, :])
            for quart in range(4):
                hs = slice(quart * (S // 4), (quart + 1) * (S // 4))
                for dc in range(NDC):
                    nc.sync.dma_start(xts[dc][:, hs], xT[dc * P:(dc + 1) * P, hs])
            nc.sync.dma_start(wosz[:, :], woS[:, :])

            for st in range(NST):
                for name in ("v", "k", "q"):
                    w = wsb[name]
                    if name == "v":
                        # V directly in [kpos, chan] layout; 4 chunks per st
                        # into one 3-D psum bank, then fp8 cast + residual
                        vps = av_ps([P, 4, P])
                        for c4 in range(4):
                            ch = st * 4 + c4
                            for dc in range(NDC):
                                nc.tensor.matmul(
                                    vps[:, c4:c4 + 1, :],
                                    xts[dc][:, ch * P:(ch + 1) * P],
                                    w[:, dc * P:(dc + 1) * P],
                                    start=(dc == 0),
                                    stop=(dc == NDC - 1),
                                )
                        cs = slice(st * 4, (st + 1) * 4)
                        for h in range(2):
                            h0 = h * DK
                            nc.scalar.copy(
                                vb8a[h][:, cs, 0:DK], vps[:, :, h0:h0 + DK]
                            )
                            # residual for dk 0:62 into the padding columns
                            nc.vector.tensor_sub(
                                vb8a[h][:, cs, DK + 1:P],
                                vps[:, :, h0:h0 + DK - 1],
                                vb8a[h][:, cs, 0:DK - 1],
                            )
                        continue
                    ps = av_ps([P, NQ])
                    for dc in range(NDC):
                        nc.tensor.matmul(
                            ps[:, :],
                            w[:, dc * P:(dc + 1) * P],
                            xts[dc][:, st * NQ:(st + 1) * NQ],
                            start=(dc == 0),
                            stop=(dc == NDC - 1),
                        )
                    sl = slice(st * NQ, (st + 1) * NQ)
                    if name == "k":
                        nc.scalar.copy(ktz[0][0:DK, sl], ps[0:DK, :])
                        nc.scalar.copy(ktz[1][DK:P, sl], ps[DK:P, :])
                    else:
                        nc.vector.tensor_copy(qtz[0][0:DK, sl], ps[0:DK, :])
                        nc.vector.tensor_copy(qtz[1][DK:P, sl], ps[DK:P, :])

            # ---- phase 2: flash attention -----------------------------------
            def emit_normalize(qc, av):
                # stage-parallel across the 4 (h,sub). PSUM rows of av[h,sub]:
                # 0:64 V8-AV, 64 denominator, 65:128 residual-AV (dk 0:62).
                # DMAs place head h's data on partitions 64h:64h+64 so the
                # stacked outtz feeds a dense K=128 W_O matmul.
                avs, dns, rcs = {}, {}, {}
                for sub in range(2):
                    for h in range(2):
                        raw = sb.tile([P, NQ], f32, tag=f"raw{h}{sub}",
                                      name=f"raw{h}{sub}", bufs=2)
                        nc.vector.tensor_copy(raw[:, :], av[h, sub][:, :])
                        avs[h, sub] = raw
                for sub in range(2):
                    for h in range(2):
                        raw = avs[h, sub]
                        b0 = h * DK
                        res = sb.tile([P, NQ], f32, tag=f"res{h}{sub}",
                                      name=f"res{h}{sub}", bufs=2)
                        dn0 = sb.tile([P, NQ], f32, tag=f"dn{h}{sub}", bufs=2)
                        nc.sync.dma_start(res[b0:b0 + DK - 1, :],
                                          raw[DK + 1:P, :])
                        nc.sync.dma_start(dn0[0:1, :], raw[DK:DK + 1, :])
                        if h == 1:
                            # move main to rows 64:128 for the stacked outtz
                            main = sb.tile([P, NQ], f32, tag=f"avm{sub}",
                                           name=f"avm{sub}", bufs=2)
                            nc.sync.dma_start(main[DK:P, :], raw[0:DK, :])
                            avs[h, sub] = main
                        dns[h, sub] = dn0
                        rcs[h, sub] = res
                for sub in range(2):
                    for h in range(2):
                        b0 = h * DK
                        # fold the residual back (in-place add, dk 0:62)
                        nc.vector.tensor_add(
                            avs[h, sub][b0:b0 + DK - 1, :],
                            avs[h, sub][b0:b0 + DK - 1, :],
                            rcs[h, sub][b0:b0 + DK - 1, :],
                        )
                        rc = sb.tile([P, NQ], f32, tag=f"rc{h}{sub}", bufs=2)
                        nc.vector.reciprocal_approx_fast(rc[0:1, :],
                                                         dns[h, sub][0:1, :])
                        rcs[h, sub] = rc
                rcbs = {}
                for sub in range(2):
                    for h in range(2):
                        rcb = sb.tile([P, NQ], f32, tag=f"rcb{h}{sub}", bufs=2)
                        nc.gpsimd.partition_broadcast(
                            rcb[:, :], rcs[h, sub][0:1, :], channels=P
                        )
                        rcbs[h, sub] = rcb
                for sub in range(2):
                    for h in range(2):
                        b0 = h * DK
                        q0 = qc * QCH + sub * NQ
                        nc.vector.tensor_mul(
                            outtz[b0:b0 + DK, q0:q0 + NQ],
                            avs[h, sub][b0:b0 + DK, :],
                            rcbs[h, sub][b0:b0 + DK, :],
                        )
                for sub in range(2):
                    emit_wo_st(2 * qc + sub)

            def emit_wo_st(st):
                for ec in range(NDC):
                    yp = av_ps([P, NQ])
                    nc.tensor.matmul(
                        yp[:, :],
                        wosz[:, ec * P:(ec + 1) * P],
                        outtz[:, st * NQ:(st + 1) * NQ],
                        start=True,
                        stop=True,
                    )
                    ys = sb.tile([P, NQ], f16, tag="ys", bufs=4)
                    if st >= NST - 2:
                        nc.vector.tensor_copy(ys[:, :], yp[:, :])
                    else:
                        nc.scalar.copy(ys[:, :], yp[:, :])
                    nc.sync.dma_start(
                        yT[ec * P:(ec + 1) * P, st * NQ:(st + 1) * NQ],
                        ys[:, :],
                    )

            pending = None
            av_pending = None

            def emit_av_pair(j, exb):
                # one DR chain per (h,sub): stationary [V8|ones|Vres] M=128
                last = j == NPAIR - 1
                for h in range(2):
                    for sub in range(2):
                        rhs = exb[h][:, :, sub * NQ:(sub + 1) * NQ]
                        nc.tensor.matmul(
                            av[h, sub][:, :], vb8a[h][:, 2 * j:2 * j + 2, :],
                            rhs, start=(j == 0), stop=last,
                            perf_mode=DR, skip_group_check=True,
                        )

            for qc in range(NQC):
                # av tiles are claimed AFTER the pending normalize's W_O yp
                # tiles (at k==3) so the tag-ring WAR chain is
                # av(qc-1) -> wo(qc-1) -> av(qc), not av(qc) -> wo(qc-1):
                # otherwise W_O(qc-1) waits a full extra qc and the whole
                # exp->AV pipeline clogs behind it at every qc boundary.
                av = {}

                def alloc_av():
                    for h in range(2):
                        for sub in range(2):
                            av[h, sub] = av_ps([P, NQ])

                if pending is None:
                    alloc_av()
                exb = None
                for k in range(NKC):
                    if k == 3 and pending is not None:
                        emit_normalize(*pending)
                        pending = None
                        alloc_av()
                    j, par = divmod(k, 2)
                    scps = [[psp.tile([P, NQ], f32, tag=f"sc{h}{sub}",
                                      name=f"sc{h}{sub}")
                             for sub in range(2)] for h in range(2)]
                    for h in range(2):
                        for sub in range(2):
                            q0 = qc * QCH + sub * NQ
                            nc.tensor.matmul(
                                scps[h][sub][:, :],
                                ktz[h][:, k * P:(k + 1) * P],
                                qtz[h][:, q0:q0 + NQ],
                                start=True,
                                stop=True,
                            )
                    if par == 0:
                        exb = {h: sb.tile([P, 2, QCH], f8, tag=f"ex{h}",
                                          name=f"ex{h}", bufs=4)
                               for h in range(2)}
                    for h in range(2):
                        exi8 = exb[h].bitcast(i8)
                        for sub in range(2):
                            ssl = slice(sub * NQ, (sub + 1) * NQ)
                            eng = pick_engine()
                            if eng == "act":
                                nc.scalar.activation(
                                    exb[h][:, par:par + 1, ssl],
                                    scps[h][sub][:, :], Act.Exp,
                                    scale=ACT_SCALE, bias=biast[:, 0:1],
                                )
                            else:
                                nc.vector.tensor_scalar(
                                    exi8[:, par:par + 1, ssl],
                                    scps[h][sub][:, :], 0.0, 119.0,
                                    Alu.max, Alu.min,
                                )
                    if par == 1:
                        if av_pending is not None:
                            emit_av_pair(*av_pending)
                        av_pending = (j, exb)
                emit_av_pair(*av_pending)
                av_pending = None
                pending = (qc, av)
            emit_normalize(*pending)

    nc.compile()
    return nc


def _wsb(w, e0, scale=1.0):
    # SBUF weight layout: w_sb[p, dc*P + m] = w[e0 + m, dc*P + p]
    wt = (w[e0:e0 + P].T * scale).astype(np.float16)     # [D, P]
    return np.ascontiguousarray(
        np.hstack([wt[d * P:(d + 1) * P, :] for d in range(NDC)])
    )


def kernel(x, wq, wk, wv, wo):
    global LAST_RESULTS
    from concourse.bass_utils import run_bass_kernel_spmd

    if "nc" not in _CACHE:
        _CACHE["nc"] = _build_nc()
    nc = _CACHE["nc"]

    x = np.asarray(x, dtype=np.float32)
    wq = np.asarray(wq, dtype=np.float32)
    wk = np.asarray(wk, dtype=np.float32)
    wv = np.asarray(wv, dtype=np.float32)
    wo = np.asarray(wo, dtype=np.float32)

    in_maps = []
    for c in range(8):
        b, hp = divmod(c, 4)
        e0 = hp * P
        in_maps.append({
            "xT": np.ascontiguousarray(x[b].T.astype(np.float16)),
            "wqS": _wsb(wq, e0),
            "wkS": _wsb(wk, e0, scale=KSCALE),
            "wvS": _wsb(wv, e0),
            "woS": np.ascontiguousarray(
                wo[:, e0:e0 + P].T.astype(np.float16)),
        })

    res = run_bass_kernel_spmd(
        nc, in_maps, core_ids=list(range(8)), trace=TRACE, tmpdir=TMPDIR
    )
    LAST_RESULTS = res

    y = np.zeros((B, S, D), dtype=np.float32)
    for c in range(8):
        y[c // 4] += res.results[c]["yT"].T.astype(np.float32)
    return y


# revision 6
# speedup vs baseline: 1.6990x; 1.0133x over previous
# Multi-head self-attention (B=2, S=4096, D=512, H=8) on 8 NeuronCores. v2.
#
# Sharding: core c -> batch b = c//4, head-pair hp = c%4 (heads 2hp, 2hp+1).
# Host pre-slices/transposes weights + x per core; device does everything;
# host sums the 4 per-core W_O partials per batch and transposes back.
#
# v2 changes vs the fp16 baseline (306 us):
#  * AV matmul in fp8e4 (IEEE e4m3) with MatmulPerfMode.DoubleRow: each
#    instruction contracts 2x128 kpos (matmul cost is N-moving-cols bound,
#    so doubling K halves the AV instruction count). Accuracy is kept with
#    an error-feedback split V = V8 + Vres8 packed into ONE stationary:
#    columns [V8(64) | ones(1) | Vres(dk 0:62)] (M=128). PSUM rows 0:63 =
#    V8-AV, row 64 = denominator, rows 65:128 = residual-AV; normalize
#    folds the residual back with a partition-shifting DMA + DVE add.
#    (dk63 keeps plain-V8 accuracy: 1 of 64 channels, ~1e-3 metric cost.)
#  * exp outputs e4m3 directly. The affine for the fp8 Schraudolph bit
#    trick is folded INTO the scores matmul: wk is pre-scaled by log2(e)
#    host-side and a constant B-row (extra contraction row) adds the bit
#    bias, so scores psum = 1.4427*s + B8. The DVE/Pool path is then just
#    round(clamp(psum, 0, 119)) -> int8 (one tensor_scalar, both ALU slots
#    used for the clamp; 119 = 0x77 = 240.0, the IEEE-e4m3 max — codes
#    120+ are inf/nan). The ACT path inverts the affine with scale/bias
#    inside the Exp activation. exp work is split ACT/DVE/Pool ~42/32/26
#    by a deficit-weighted round-robin to balance engine busy time.
#  * scores/projections/W_O stay fp16 (fp8 there fails the error budget;
#    DR+residual for scores costs more than fp16).
#  * yT output in fp16 (host accumulates partials in fp32).

import numpy as np

B, S, D, H, DK = 2, 4096, 512, 8, 64
P = 128          # partition tile
NQ = 512         # matmul moving free dim (one fp32 PSUM bank)
QCH = 1024       # q-chunk (2 x NQ)
NKC = S // P     # kpos chunks (32)
NPAIR = NKC // 2  # DoubleRow kpos chunk-pairs (16)
NST = S // NQ    # s-tiles of 512 (8)
NDC = D // P     # d chunks (4)
NQC = S // QCH   # q chunks (4)

# exp(s/8 - SHIFT) in e4m3: bits = round(clamp(KS*s + B8, 0, 119)).
# KS is folded into wk host-side; B8 enters via a constant contraction row.
SHIFT = 2.0
SCH_C = 0.055                      # centers the linear-mantissa error
KSCALE = 1.4426950408889634        # 8*log2(e)/8
A8 = 8.0 * KSCALE                  # e4m3 codes per e-fold
B8 = 56.0 - 8.0 * SCH_C - A8 * SHIFT       # 32.4768792...
ACT_SCALE = 1.0 / A8               # invert the affine for the true-exp path
ACT_BIAS = -(B8 / A8 + SHIFT)      # exp(psum*ACT_SCALE + ACT_BIAS)

# exp engine split (deficit-weighted round-robin). GPSIMD cannot read PSUM,
# so only ACT/DVE can consume the scores; Pool gets SBUF-only side work.
ENG_FRAC = {"act": 0.557, "dve": 0.443}

TRACE = False            # test.py sets True to get exec_time_ns + perfetto
TMPDIR = None            # optional trace output dir
LAST_RESULTS = None      # BassKernelResults of the last run (for test.py)
EXP_ASSIGN = []          # engine per exp op, emission order (for sim model)

_CACHE = {}


def _build_nc():
    import concourse.bass as bass  # noqa: F401
    import concourse.mybir as mybir
    import concourse.tile as tile
    from concourse import bacc

    f32 = mybir.dt.float32
    f16 = mybir.dt.float16
    f8 = mybir.dt.float8e4
    i8 = mybir.dt.int8
    Act = mybir.ActivationFunctionType
    Alu = mybir.AluOpType
    DR = mybir.MatmulPerfMode.DoubleRow

    nc = bacc.Bacc("TRN2", target_bir_lowering=False, debug=False, num_devices=8)

    xT = nc.dram_tensor("xT", [D, S], f16, kind="ExternalInput")
    wqS = nc.dram_tensor("wqS", [P, D], f16, kind="ExternalInput")
    wkS = nc.dram_tensor("wkS", [P, D], f16, kind="ExternalInput")  # *KSCALE
    wvS = nc.dram_tensor("wvS", [P, D], f16, kind="ExternalInput")
    woS = nc.dram_tensor("woS", [P, D], f16, kind="ExternalInput")  # stacked
    yT = nc.dram_tensor("yT", [D, S], f16, kind="ExternalOutput")

    # deficit round-robin over the 512 exp ops
    eng_done = {k: 0.0 for k in ENG_FRAC}
    eng_n = [0]

    def pick_engine():
        eng_n[0] += 1
        best, bestd = None, None
        for k, f in ENG_FRAC.items():
            d = f * eng_n[0] - eng_done[k]
            if bestd is None or d > bestd:
                best, bestd = k, d
        eng_done[best] += 1.0
        EXP_ASSIGN.append(best)
        return best

    with tile.TileContext(nc) as tc:
        with (
            tc.tile_pool(name="sb", bufs=1) as sb,
            tc.tile_pool(name="ps", bufs=1, space="PSUM") as psp,
        ):
            # PSUM budget (8 banks): sc{h}{sub} 4 x [128,512] + av/phase1/WO
            # slots av0..av3 shared by tag rotation.
            psn = 0

            def av_ps(shape):
                nonlocal psn
                t = psp.tile(shape, f32, tag=f"av{psn % 4}", name=f"avps{psn}")
                psn += 1
                return t

            # ---- persistent operand tiles -----------------------------------
            qtz = [sb.tile([P, S], f16, tag=f"qtz{h}", name=f"qtz{h}")
                   for h in range(2)]
            ktz = [sb.tile([P, S], f16, tag=f"ktz{h}", name=f"ktz{h}")
                   for h in range(2)]
            # AV stationary: [kpos, chunk, (V8 dk 0:64 | ones | Vres dk 0:63)]
            vb8a = [sb.tile([P, NKC, P], f8, tag=f"vb8a{h}", name=f"vb8a{h}")
                    for h in range(2)]
            # stacked: rows 0:64 head0, rows 64:128 head1
            outtz = sb.tile([P, S], f16, tag="outtz", name="outtz")
            wosz = sb.tile([P, D], f16, tag="wosz", name="wosz")

            # head h=0 lives on partitions 0:64 (B-row at 64, zeros above),
            # head h=1 on partitions 64:128 (B-row at 0, zeros below).
            # Bands split DVE/gpsimd: both are idle during the initial x DMA,
            # and all bands must be done before the first scores matmul.
            nc.vector.memset(qtz[0][DK:P, :], 0.0)
            nc.vector.memset(ktz[0][DK:P, :], 0.0)
            nc.gpsimd.memset(qtz[1][0:DK, :], 0.0)
            nc.gpsimd.memset(ktz[1][0:DK, :], 0.0)
            nc.gpsimd.memset(qtz[0][DK:DK + 1, :], 1.0)
            nc.gpsimd.memset(ktz[0][DK:DK + 1, :], B8)
            nc.gpsimd.memset(qtz[1][0:1, :], 1.0)
            nc.gpsimd.memset(ktz[1][0:1, :], B8)
            for h in range(2):
                # ones col 64 (V8/Vres cols written by phase 1)
                nc.gpsimd.memset(vb8a[h][:, :, DK:DK + 1], 1.0)

            # bias const for the ACT exp path; ones row for the PE broadcast
            biast = sb.tile([P, 1], f32, tag="biast", name="biast")
            nc.gpsimd.memset(biast[:, :], ACT_BIAS)
            ones16 = sb.tile([1, P], f16, tag="ones16", name="ones16")
            nc.gpsimd.memset(ones16[:, :], 1.0)

            # ---- phase 1: load x + weights, QKV projections, build V -------
            xts = [sb.tile([P, S], f16, tag=f"xt{dc}", name=f"xt{dc}")
                   for dc in range(NDC)]
            wsb = {}
            for name in ("v", "k", "q"):
                wsb[name] = sb.tile([P, NDC * P], f16, tag=f"w{name}",
                                    name=f"w{name}")
            # weights first (small) so the first projection matmul isn't
            # gated behind the 4MB x stream; then x quarter-tiles
            for name, dram in (("v", wvS), ("k", wkS), ("q", wqS)):
                nc.sync.dma_start(wsb[name][:, :], dram[:, :])
            for quart in range(4):
                hs = slice(quart * (S // 4), (quart + 1) * (S // 4))
                for dc in range(NDC):
                    nc.sync.dma_start(xts[dc][:, hs], xT[dc * P:(dc + 1) * P, hs])
            nc.sync.dma_start(wosz[:, :], woS[:, :])

            for st in range(NST):
                for name in ("v", "k", "q"):
                    w = wsb[name]
                    if name == "v":
                        # V directly in [kpos, chan] layout; 4 chunks per st
                        # into one 3-D psum bank, then fp8 cast + residual
                        vps = av_ps([P, 4, P])
                        for c4 in range(4):
                            ch = st * 4 + c4
                            for dc in range(NDC):
                                nc.tensor.matmul(
                                    vps[:, c4:c4 + 1, :],
                                    xts[dc][:, ch * P:(ch + 1) * P],
                                    w[:, dc * P:(dc + 1) * P],
                                    start=(dc == 0),
                                    stop=(dc == NDC - 1),
                                )
                        cs = slice(st * 4, (st + 1) * 4)
                        for h in range(2):
                            h0 = h * DK
                            nc.scalar.copy(
                                vb8a[h][:, cs, 0:DK], vps[:, :, h0:h0 + DK]
                            )
                            # residual for dk 0:62 into the padding columns
                            nc.vector.tensor_sub(
                                vb8a[h][:, cs, DK + 1:P],
                                vps[:, :, h0:h0 + DK - 1],
                                vb8a[h][:, cs, 0:DK - 1],
                            )
                        continue
                    ps = av_ps([P, NQ])
                    for dc in range(NDC):
                        nc.tensor.matmul(
                            ps[:, :],
                            w[:, dc * P:(dc + 1) * P],
                            xts[dc][:, st * NQ:(st + 1) * NQ],
                            start=(dc == 0),
                            stop=(dc == NDC - 1),
                        )
                    sl = slice(st * NQ, (st + 1) * NQ)
                    if name == "k":
                        nc.scalar.copy(ktz[0][0:DK, sl], ps[0:DK, :])
                        nc.scalar.copy(ktz[1][DK:P, sl], ps[DK:P, :])
                    else:
                        nc.vector.tensor_copy(qtz[0][0:DK, sl], ps[0:DK, :])
                        nc.vector.tensor_copy(qtz[1][DK:P, sl], ps[DK:P, :])

            # ---- phase 2: flash attention -----------------------------------
            def emit_normalize(qc, av):
                # stage-parallel across the 4 (h,sub). PSUM rows of av[h,sub]:
                # 0:64 V8-AV, 64 denominator, 65:128 residual-AV (dk 0:62).
                # DMAs place head h's data on partitions 64h:64h+64 so the
                # stacked outtz feeds a dense K=128 W_O matmul.
                avs, dns, rcs = {}, {}, {}
                for sub in range(2):
                    for h in range(2):
                        raw = sb.tile([P, NQ], f32, tag=f"raw{h}{sub}",
                                      name=f"raw{h}{sub}", bufs=2)
                        nc.vector.tensor_copy(raw[:, :], av[h, sub][:, :])
                        avs[h, sub] = raw
                for sub in range(2):
                    for h in range(2):
                        raw = avs[h, sub]
                        b0 = h * DK
                        res = sb.tile([P, NQ], f32, tag=f"res{h}{sub}",
                                      name=f"res{h}{sub}", bufs=2)
                        dn0 = sb.tile([P, NQ], f32, tag=f"dn{h}{sub}", bufs=2)
                        nc.sync.dma_start(res[b0:b0 + DK - 1, :],
                                          raw[DK + 1:P, :])
                        nc.sync.dma_start(dn0[0:1, :], raw[DK:DK + 1, :])
                        if h == 1:
                            # move main to rows 64:128 for the stacked outtz
                            main = sb.tile([P, NQ], f32, tag=f"avm{sub}",
                                           name=f"avm{sub}", bufs=2)
                            nc.sync.dma_start(main[DK:P, :], raw[0:DK, :])
                            avs[h, sub] = main
                        dns[h, sub] = dn0
                        rcs[h, sub] = res
                for sub in range(2):
                    for h in range(2):
                        b0 = h * DK
                        # fold the residual back (in-place add, dk 0:62)
                        nc.vector.tensor_add(
                            avs[h, sub][b0:b0 + DK - 1, :],
                            avs[h, sub][b0:b0 + DK - 1, :],
                            rcs[h, sub][b0:b0 + DK - 1, :],
                        )
                        rc = sb.tile([P, NQ], f32, tag=f"rc{h}{sub}", bufs=2)
                        nc.vector.reciprocal_approx_fast(rc[0:1, :],
                                                         dns[h, sub][0:1, :])
                        rch = sb.tile([P, NQ], f16, tag=f"rch{h}{sub}",
                                      bufs=2)
                        nc.vector.tensor_copy(rch[0:1, :], rc[0:1, :])
                        rcs[h, sub] = rch
                # broadcast 1/den across partitions on the PE (rank-1
                # matmul) — keeps gpsimd (slow dispatch) off this path
                rcbs = {}
                for sub in range(2):
                    for h in range(2):
                        rcb = av_ps([P, NQ])
                        nc.tensor.matmul(
                            rcb[:, :], ones16[0:1, :], rcs[h, sub][0:1, :],
                            start=True, stop=True,
                        )
                        rcbs[h, sub] = rcb
                for sub in range(2):
                    for h in range(2):
                        b0 = h * DK
                        q0 = qc * QCH + sub * NQ
                        nc.vector.tensor_mul(
                            outtz[b0:b0 + DK, q0:q0 + NQ],
                            avs[h, sub][b0:b0 + DK, :],
                            rcbs[h, sub][b0:b0 + DK, :],
                        )
                    emit_wo_st(2 * qc + sub)

            def emit_wo_st(st):
                for ec in range(NDC):
                    yp = av_ps([P, NQ])
                    nc.tensor.matmul(
                        yp[:, :],
                        wosz[:, ec * P:(ec + 1) * P],
                        outtz[:, st * NQ:(st + 1) * NQ],
                        start=True,
                        stop=True,
                    )
                    ys = sb.tile([P, NQ], f16, tag="ys", bufs=4)
                    if st >= NST - 2:
                        nc.vector.tensor_copy(ys[:, :], yp[:, :])
                    else:
                        nc.scalar.copy(ys[:, :], yp[:, :])
                    nc.sync.dma_start(
                        yT[ec * P:(ec + 1) * P, st * NQ:(st + 1) * NQ],
                        ys[:, :],
                    )

            pending = None
            av_pending = None

            def emit_av_pair(j, exb):
                # one DR chain per (h,sub): stationary [V8|ones|Vres] M=128
                last = j == NPAIR - 1
                for h in range(2):
                    for sub in range(2):
                        rhs = exb[h][:, :, sub * NQ:(sub + 1) * NQ]
                        nc.tensor.matmul(
                            av[h, sub][:, :], vb8a[h][:, 2 * j:2 * j + 2, :],
                            rhs, start=(j == 0), stop=last,
                            perf_mode=DR, skip_group_check=True,
                        )

            for qc in range(NQC):
                # av tiles are claimed AFTER the pending normalize's W_O yp
                # tiles (at k==3) so the tag-ring WAR chain is
                # av(qc-1) -> wo(qc-1) -> av(qc), not av(qc) -> wo(qc-1):
                # otherwise W_O(qc-1) waits a full extra qc and the whole
                # exp->AV pipeline clogs behind it at every qc boundary.
                av = {}

                def alloc_av():
                    for h in range(2):
                        for sub in range(2):
                            av[h, sub] = av_ps([P, NQ])

                if pending is None:
                    alloc_av()
                exb = None
                for k in range(NKC):
                    if k == 3 and pending is not None:
                        emit_normalize(*pending)
                        pending = None
                        alloc_av()
                    j, par = divmod(k, 2)
                    scps = [[psp.tile([P, NQ], f32, tag=f"sc{h}{sub}",
                                      name=f"sc{h}{sub}")
                             for sub in range(2)] for h in range(2)]
                    for h in range(2):
                        for sub in range(2):
                            q0 = qc * QCH + sub * NQ
                            nc.tensor.matmul(
                                scps[h][sub][:, :],
                                ktz[h][:, k * P:(k + 1) * P],
                                qtz[h][:, q0:q0 + NQ],
                                start=True,
                                stop=True,
                            )
                    if par == 0:
                        exb = {h: sb.tile([P, 2, QCH], f8, tag=f"ex{h}",
                                          name=f"ex{h}", bufs=4)
                               for h in range(2)}
                    for h in range(2):
                        exi8 = exb[h].bitcast(i8)
                        for sub in range(2):
                            ssl = slice(sub * NQ, (sub + 1) * NQ)
                            eng = pick_engine()
                            if eng == "act":
                                nc.scalar.activation(
                                    exb[h][:, par:par + 1, ssl],
                                    scps[h][sub][:, :], Act.Exp,
                                    scale=ACT_SCALE, bias=biast[:, 0:1],
                                )
                            else:
                                nc.vector.tensor_scalar(
                                    exi8[:, par:par + 1, ssl],
                                    scps[h][sub][:, :], 0.0, 119.0,
                                    Alu.max, Alu.min,
                                )
                    if par == 1:
                        if av_pending is not None:
                            emit_av_pair(*av_pending)
                        av_pending = (j, exb)
                emit_av_pair(*av_pending)
                av_pending = None
                pending = (qc, av)
            emit_normalize(*pending)

    nc.compile()
    return nc


def _wsb(w, e0, scale=1.0):
    # SBUF weight layout: w_sb[p, dc*P + m] = w[e0 + m, dc*P + p]
    wt = (w[e0:e0 + P].T * scale).astype(np.float16)     # [D, P]
    return np.ascontiguousarray(
        np.hstack([wt[d * P:(d + 1) * P, :] for d in range(NDC)])
    )


def kernel(x, wq, wk, wv, wo):
    global LAST_RESULTS
    from concourse.bass_utils import run_bass_kernel_spmd

    if "nc" not in _CACHE:
        _CACHE["nc"] = _build_nc()
    nc = _CACHE["nc"]

    x = np.asarray(x, dtype=np.float32)
    wq = np.asarray(wq, dtype=np.float32)
    wk = np.asarray(wk, dtype=np.float32)
    wv = np.asarray(wv, dtype=np.float32)
    wo = np.asarray(wo, dtype=np.float32)

    in_maps = []
    for c in range(8):
        b, hp = divmod(c, 4)
        e0 = hp * P
        in_maps.append({
            "xT": np.ascontiguousarray(x[b].T.astype(np.float16)),
            "wqS": _wsb(wq, e0),
            "wkS": _wsb(wk, e0, scale=KSCALE),
            "wvS": _wsb(wv, e0),
            "woS": np.ascontiguousarray(
                wo[:, e0:e0 + P].T.astype(np.float16)),
        })

    res = run_bass_kernel_spmd(
        nc, in_maps, core_ids=list(range(8)), trace=TRACE, tmpdir=TMPDIR
    )
    LAST_RESULTS = res

    y = np.zeros((B, S, D), dtype=np.float32)
    for c in range(8):
        y[c // 4] += res.results[c]["yT"].T.astype(np.float32)
    return y
